# revision 9
# baseline (speedup 1.0000x reference)
"""Trainium2 Bass kernel for MultiHeadLatentAttention.

Reference computation (B=2, S=2048, HIDDEN=2048, 16 heads x 128, LATENT=512):
  q_lat = x @ Wq_d ; kv_lat = x @ Wkv_d
  q = split_heads(q_lat @ Wq_u) ; k = split_heads(kv_lat @ Wk_u) ; v = split_heads(kv_lat @ Wv_u)
  q, k = rope(q, k)
  out = softmax(causal(q k^T / sqrt(d))) @ v   -> merge heads -> @ Wo

Sharding: 8 cores = 2 batches (data parallel) x 4-way tensor parallel over
heads (4 heads/core).  Each core computes the full latents for its batch
(replicated within the 4-core group), the up-projections + attention for its
4 heads, and a partial output projection over its heads' slice of Wo's input
dim.  The host sums the 4 partials per batch (cheap elementwise add).

Dataflow on-core is fully transposed ([feature, seq] layout) so no PE
transposes are needed anywhere:
  latT = Wd^T xT -> qT/kT per head via up-proj; rotate_half for rope is a
  single signed-permutation matmul on the PE; v in [seq, d] layout;
  scoresT[k, q] = kT-block-stationary x qT-moving; exp on ACT; softmax
  denominators via ones-vector matmuls accumulated on the PE; AV accumulated
  as v^T-stationary x expT; 1/denominator applied on the attention output
  (PSUM->SBUF copy fused); final Wo stage back in [seq, out] orientation.
  Causal structure skips above-diagonal blocks and narrows partial blocks.

Matmuls run in float32r (full PE rate; fp32 is 1/4 rate), fp32 accumulation.
"""

import sys
from contextlib import ExitStack

sys.path.insert(0, "/opt/trn_rl_repo")

import numpy as np

import concourse.bass as bass
import concourse.mybir as mybir
import concourse.tile as tile
from concourse import bacc
from concourse.bass_utils import run_bass_kernel_spmd

HIDDEN = 2048
LATENT = 512
NUM_HEADS = 16
HEAD_DIM = 128
THETA = 10000.0
B = 2
S_FULL = 2048
N_CORES = 8
TP = 4  # tensor-parallel group size (heads 16 / 4 = 4 per core)
HPC = NUM_HEADS // TP  # heads per core
DSL = HPC * HEAD_DIM  # per-core head-dim slice width (512)

F32 = mybir.dt.float32
F32R = mybir.dt.float32r

NEG = -1.0e30
SCALE = 1.0 / np.sqrt(HEAD_DIM)


def build_nc(S=S_FULL, finalize=True, iters=1):
    """Build the single-core SPMD program (same program all 8 cores).

    iters > 1 wraps the whole body in an on-device repeat loop (timing rig).
    """
    nc = bacc.Bacc(None, target_bir_lowering=False)

    KC_H = HIDDEN // 128   # 16 contraction chunks for hidden dim
    KC_L = LATENT // 128   # 4 contraction chunks for latent dim
    NB = S // 512          # number of 512-wide seq blocks
    SC = S // 128          # number of 128-wide seq chunks
    NH = S // 256          # number of 256-wide seq half-blocks (stage A)

    xT = nc.dram_tensor("xT", [HIDDEN, S], F32R, kind="ExternalInput")
    wqd = nc.dram_tensor("wqd", [HIDDEN, LATENT], F32R, kind="ExternalInput")
    wkvd = nc.dram_tensor("wkvd", [HIDDEN, LATENT], F32R, kind="ExternalInput")
    wqu = nc.dram_tensor("wqu", [LATENT, DSL], F32R, kind="ExternalInput")
    wku = nc.dram_tensor("wku", [LATENT, DSL], F32R, kind="ExternalInput")
    wvu = nc.dram_tensor("wvu", [LATENT, DSL], F32R, kind="ExternalInput")
    wo = nc.dram_tensor("wo", [DSL, HIDDEN], F32R, kind="ExternalInput")
    cosd = nc.dram_tensor("cosd", [128, S], F32, kind="ExternalInput")
    sind = nc.dram_tensor("sind", [128, S], F32, kind="ExternalInput")
    mtd = nc.dram_tensor("mtd", [2, 128, 256], F32, kind="ExternalInput")
    permd = nc.dram_tensor("permd", [128, 128], F32R, kind="ExternalInput")
    onesd = nc.dram_tensor("onesd", [128, 1], F32R, kind="ExternalInput")
    y = nc.dram_tensor("y", [S, HIDDEN], F32, kind="ExternalOutput")

    with tile.TileContext(nc) as tc, ExitStack() as _es:
        if iters > 1:
            _es.enter_context(tc.For_i(0, iters, 1))
        # ---- persistent pools (allocated bottom of stack, live long) ----
        with tc.tile_pool(name="p_out", bufs=1) as p_out, \
             tc.tile_pool(name="p_lat", bufs=1) as p_lat, \
             tc.tile_pool(name="p_const", bufs=1) as p_const:

            outT = p_out.tile([128, HPC, S], F32R)      # attention out, transposed
            latq = p_lat.tile([128, KC_L, S], F32R)     # q_latT
            latkv = p_lat.tile([128, KC_L, S], F32R)    # kv_latT
            mask_sb = p_const.tile([128, 2, 256], F32)
            ones_sb = p_const.tile([128, 1], F32R)
            perm_sb = p_const.tile([128, 128], F32R)

            # ================= stage A: down projections =================
            with tc.tile_pool(name="p_wd", bufs=1) as p_wd, \
                 tc.tile_pool(name="p_xt", bufs=2) as p_xt, \
                 tc.tile_pool(name="ps_a", bufs=4, space="PSUM") as ps_a:
                wqd_sb = p_wd.tile([128, KC_H, LATENT], F32R)
                wkvd_sb = p_wd.tile([128, KC_H, LATENT], F32R)

                def load_w_col(w_sb, w_dram, m):
                    nc.sync.dma_start(
                        out=w_sb[:, :, m * 128:(m + 1) * 128],
                        in_=w_dram.rearrange("(kc p) l -> p kc l", p=128)
                        [:, :, m * 128:(m + 1) * 128])

                # prefetch order: wqd m=0 first, then slab 0 (in loop), then
                # the rest, so the PE can start ~15us in.
                load_w_col(wqd_sb, wqd, 0)
                for nh in range(NH):
                    xslab = p_xt.tile([128, KC_H, 256], F32R, tag="xslab")
                    nc.sync.dma_start(
                        out=xslab,
                        in_=xT.rearrange("(kc p) s -> p kc s", p=128)
                        [:, :, nh * 256:(nh + 1) * 256])
                    if nh == 0:
                        load_w_col(wkvd_sb, wkvd, 0)
                        for m in range(1, KC_L):
                            load_w_col(wqd_sb, wqd, m)
                            load_w_col(wkvd_sb, wkvd, m)
                        # constants (needed later; low priority)
                        nc.sync.dma_start(
                            out=mask_sb, in_=mtd.rearrange("j p c -> p j c"))
                        nc.sync.dma_start(out=ones_sb, in_=onesd[:, :])
                        nc.sync.dma_start(out=perm_sb, in_=permd[:, :])
                    for w_sb, lat in ((wqd_sb, latq), (wkvd_sb, latkv)):
                        for m in range(KC_L):
                            acc = ps_a.tile([128, 256], F32, tag="acc_a")
                            for kc in range(KC_H):
                                nc.tensor.matmul(
                                    acc,
                                    w_sb[:, kc, m * 128:(m + 1) * 128],
                                    xslab[:, kc, :],
                                    start=(kc == 0), stop=(kc == KC_H - 1))
                            nc.scalar.copy(
                                lat[:, m, nh * 256:(nh + 1) * 256], acc)

            # ================= stage B0: v for all 4 heads ===============
            with tc.tile_pool(name="p_v", bufs=1) as p_v:
                with tc.tile_pool(name="p_wv", bufs=1) as p_wv, \
                     tc.tile_pool(name="ps_v", bufs=2, space="PSUM") as ps_v:
                    v_sb = p_v.tile([128, SC, DSL], F32R)
                    wvu_sb = p_wv.tile([128, KC_L, DSL], F32R)
                    nc.sync.dma_start(
                        out=wvu_sb,
                        in_=wvu.rearrange("(kc p) d -> p kc d", p=128))
                    for sc in range(SC):
                        acc = ps_v.tile([128, DSL], F32, tag="acc_v")
                        for kc in range(KC_L):
                            nc.tensor.matmul(
                                acc,
                                latkv[:, kc, sc * 128:(sc + 1) * 128],
                                wvu_sb[:, kc, :],
                                start=(kc == 0), stop=(kc == KC_L - 1))
                        nc.vector.tensor_copy(v_sb[:, sc, :], acc)

                # ============ stages B/C per head: up-proj + attention ====
                with tc.tile_pool(name="p_rope", bufs=1) as p_rope:
                    cos_sb = p_rope.tile([128, S], F32)
                    sin_sb = p_rope.tile([128, S], F32)
                    nc.sync.dma_start(out=cos_sb, in_=cosd[:, :])
                    nc.sync.dma_start(out=sin_sb, in_=sind[:, :])
                    with tc.tile_pool(name="ps_b", bufs=2, space="PSUM") as ps_b, \
                         tc.tile_pool(name="ps_br", bufs=1, space="PSUM") as ps_br, \
                         tc.tile_pool(name="ps_s", bufs=2, space="PSUM") as ps_s, \
                         tc.tile_pool(name="ps_o", bufs=2, space="PSUM") as ps_o, \
                         tc.tile_pool(name="ps_n", bufs=1, space="PSUM") as ps_n, \
                         tc.tile_pool(name="p_rt", bufs=3) as p_rt, \
                         tc.tile_pool(name="p_at", bufs=3) as p_at, \
                         tc.tile_pool(name="p_rb", bufs=2) as p_rb:
                      for h in range(HPC):
                        with tc.tile_pool(name="p_head", bufs=1) as p_head, \
                             tc.tile_pool(name="p_wu", bufs=2) as p_wu:
                            qT = p_head.tile([128, S], F32R, tag="qT")
                            kT = p_head.tile([128, S], F32R, tag="kT")
                            wq_sb = p_wu.tile([128, KC_L, 128], F32R, tag="wq")
                            wk_sb = p_wu.tile([128, KC_L, 128], F32R, tag="wk")
                            hs = h * 128
                            nc.sync.dma_start(
                                out=wq_sb,
                                in_=wqu.rearrange("(kc p) d -> p kc d", p=128)
                                [:, :, hs:hs + 128])
                            nc.sync.dma_start(
                                out=wk_sb,
                                in_=wku.rearrange("(kc p) d -> p kc d", p=128)
                                [:, :, hs:hs + 128])

                            for dst, w_sb, lat in (
                                    (qT, wq_sb, latq),
                                    (kT, wk_sb, latkv)):
                                for nb in range(NB):
                                    sl = slice(nb * 512, (nb + 1) * 512)
                                    pa = ps_b.tile([128, 512], F32, tag="pa")
                                    for kc in range(KC_L):
                                        nc.tensor.matmul(
                                            pa, w_sb[:, kc, :], lat[:, kc, sl],
                                            start=(kc == 0),
                                            stop=(kc == KC_L - 1))
                                    raw = p_rt.tile([128, 512], F32R, tag="raw")
                                    nc.vector.tensor_copy(raw, pa)
                                    pr = ps_br.tile([128, 512], F32, tag="pr")
                                    nc.tensor.matmul(pr, perm_sb, raw,
                                                     start=True, stop=True)
                                    rt = p_rt.tile([128, 512], F32, tag="rt")
                                    nc.vector.tensor_mul(dst[:, sl], pa,
                                                         cos_sb[:, sl])
                                    nc.vector.tensor_mul(rt, pr, sin_sb[:, sl])
                                    nc.vector.tensor_add(dst[:, sl],
                                                         dst[:, sl], rt)

                            # ---- attention for head h ----
                            if True:
                                for qb in range(NB):
                                    kb_hi = 4 * qb + 4
                                    po = ps_o.tile([128, 512], F32, tag="po")
                                    pn = ps_n.tile([1, 512], F32, tag="pn")
                                    for kb in range(kb_hi):
                                        j = kb - 4 * qb
                                        # narrowed q range for partial blocks
                                        # (keep N >= 256 for the f32r rate)
                                        off = min(j, 2) * 128 if j >= 0 else 0
                                        w = 512 - off
                                        q0 = qb * 512 + off
                                        ps = ps_s.tile([128, 512], F32, tag="ps")
                                        nc.tensor.matmul(
                                            ps[:, 0:w],
                                            kT[:, kb * 128:(kb + 1) * 128],
                                            qT[:, q0:q0 + w],
                                            start=True, stop=True)
                                        if j >= 0:
                                            jj = j - off // 128
                                            mw = (jj + 1) * 128
                                            nc.vector.tensor_add(
                                                ps[:, 0:mw], ps[:, 0:mw],
                                                mask_sb[:, jj, 0:mw])
                                        et = p_at.tile([128, 512], F32R, tag="et")
                                        nc.scalar.activation(
                                            out=et[:, 0:w], in_=ps[:, 0:w],
                                            func=mybir.ActivationFunctionType.Exp,
                                            scale=float(SCALE))
                                        nc.tensor.matmul(
                                            po[:, off:512],
                                            v_sb[:, kb, hs:hs + 128],
                                            et[:, 0:w],
                                            start=(kb == 0),
                                            stop=(kb == kb_hi - 1))
                                        nc.tensor.matmul(
                                            pn[0:1, off:512],
                                            ones_sb[:, 0:1],
                                            et[:, 0:w],
                                            start=(kb == 0),
                                            stop=(kb == kb_hi - 1))
                                    qsl = slice(qb * 512, (qb + 1) * 512)
                                    rc = p_rb.tile([1, 512], F32, tag="rc")
                                    nc.vector.reciprocal(rc, pn[0:1, :])
                                    rb = p_rb.tile([128, 512], F32, tag="rb")
                                    nc.gpsimd.partition_broadcast(rb, rc)
                                    nc.vector.tensor_mul(outT[:, h, qsl], po, rb)

            # ================= stage D: output projection ================
            with tc.tile_pool(name="p_wo", bufs=1) as p_wo, \
                 tc.tile_pool(name="p_fin", bufs=3) as p_fin, \
                 tc.tile_pool(name="ps_d", bufs=4, space="PSUM") as ps_d:
                wo_sb = p_wo.tile([128, HPC, HIDDEN], F32R, tag="wo")
                nc.sync.dma_start(
                    out=wo_sb, in_=wo.rearrange("(ic p) o -> p ic o", p=128))
                for sc in range(SC):
                    fin = p_fin.tile([128, HIDDEN], F32, tag="fin")
                    for ob in range(HIDDEN // 512):
                        acc = ps_d.tile([128, 512], F32, tag="acc_d")
                        for ic in range(HPC):
                            nc.tensor.matmul(
                                acc,
                                outT[:, ic, sc * 128:(sc + 1) * 128],
                                wo_sb[:, ic, ob * 512:(ob + 1) * 512],
                                start=(ic == 0), stop=(ic == HPC - 1))
                        osl = slice(ob * 512, (ob + 1) * 512)
                        if ob % 2 == 0:
                            nc.scalar.copy(fin[:, osl], acc)
                        else:
                            nc.vector.tensor_copy(fin[:, osl], acc)
                    nc.sync.dma_start(
                        out=y[sc * 128:(sc + 1) * 128, :], in_=fin)

    if finalize:
        nc.finalize()
    return nc


# ---------------------------------------------------------------------------
# host-side helpers


def host_inputs(x, Wq_d, Wkv_d, Wq_u, Wk_u, Wv_u, Wo, S=S_FULL):
    """Build the 8 per-core input maps from full inputs."""
    x = np.asarray(x, dtype=np.float32)
    Wq_d = np.asarray(Wq_d, dtype=np.float32)
    Wkv_d = np.asarray(Wkv_d, dtype=np.float32)
    Wq_u = np.asarray(Wq_u, dtype=np.float32)
    Wk_u = np.asarray(Wk_u, dtype=np.float32)
    Wv_u = np.asarray(Wv_u, dtype=np.float32)
    Wo = np.asarray(Wo, dtype=np.float32)

    inv_freq = 1.0 / (THETA ** (np.arange(0, HEAD_DIM, 2, dtype=np.float64)
                                / HEAD_DIM))  # (64,)
    pos = np.arange(S, dtype=np.float64)
    ang = pos[None, :] * np.concatenate([inv_freq, inv_freq])[:, None]  # (128, S)
    COS = np.cos(ang).astype(np.float32)
    SIN = np.sin(ang).astype(np.float32)

    # masks for narrowed diagonal blocks: jj=0 -> mask c<r on first 128 cols;
    # jj=1 -> mask c<128+r on first 256 cols
    MT = np.zeros((2, 128, 256), dtype=np.float32)
    r = np.arange(128)[:, None]
    c = np.arange(256)[None, :]
    MT[0] = np.where(c >= r, 0.0, NEG)
    MT[1] = np.where(c >= 128 + r, 0.0, NEG)

    # signed permutation for rotate_half in [d, seq] layout:
    # out[m] = -in[m+64] for m<64 ; +in[m-64] for m>=64
    PERM = np.zeros((128, 128), dtype=np.float32)
    for m in range(64):
        PERM[m + 64, m] = -1.0
        PERM[m, m + 64] = 1.0

    in_maps = []
    for core in range(N_CORES):
        b, tp = core // TP, core % TP
        sl = slice(tp * DSL, (tp + 1) * DSL)
        in_maps.append({
            "xT": np.ascontiguousarray(x[b, :S].T),
            "wqd": Wq_d,
            "wkvd": Wkv_d,
            "wqu": np.ascontiguousarray(Wq_u[:, sl]),
            "wku": np.ascontiguousarray(Wk_u[:, sl]),
            "wvu": np.ascontiguousarray(Wv_u[:, sl]),
            "wo": np.ascontiguousarray(Wo[sl, :]),
            "cosd": COS,
            "sind": SIN,
            "mtd": MT,
            "permd": PERM,
            "onesd": np.ones((128, 1), dtype=np.float32),
        })
    return in_maps


def assemble(results, S=S_FULL):
    out = np.zeros((B, S, HIDDEN), dtype=np.float32)
    for core in range(N_CORES):
        out[core // TP] += results[core]["y"]
    return out


_NC_CACHE = {}


def kernel(x, Wq_d, Wkv_d, Wq_u, Wk_u, Wv_u, Wo):
    S = x.shape[1]
    if S not in _NC_CACHE:
        _NC_CACHE[S] = build_nc(S)
    nc = _NC_CACHE[S]
    in_maps = host_inputs(x, Wq_d, Wkv_d, Wq_u, Wk_u, Wv_u, Wo, S=S)
    res = run_bass_kernel_spmd(nc, in_maps, list(range(N_CORES)))
    return assemble(res.results, S=S)
